# revision 6
# baseline (speedup 1.0000x reference)
"""Distributed attention kernel for one TRN2 chip (8 NeuronCores).

Problem: multi-head cross-attention
  B=4, TQ=512, TKV=4096, D=1024, H=8 heads (head_dim=128)

Sharding (data-parallel x tensor-parallel, per the hint):
  core c in 0..7 -> (batch b = c % 4, head-group g = c // 4)
  Each core computes heads [4g, 4g+4) for its batch (Wq/Wk/Wv column
  shards). Per-head U results are pair-exchanged (c <-> c+4) with an
  AllGather DURING the attention phase, so every core ends with all 8
  heads' U and computes its own 512-column slice of the output
  projection locally - no serialized collective tail.

Numerics / PE-work tricks (all bf16-accuracy or better):
  - Projections run as fp8e4m3 "DoubleRow" matmuls (2x128 contraction
    rows per pass = 2x bf16 column rate). Host splits x and 64*W into
    (hi, lo) fp8 pairs; the 3-term scheme
        W^T x ~= Whi^T xhi + Wlo^T xhi + Whi^T xlo
    costs 0.75x the bf16 cycles and only drops the Wlo^T xlo term
    (~2^-8 relative, below bf16 input-rounding noise).
  - W is pre-scaled by 64 so its lo-half stays out of fp8-subnormal
    range; Q^T/K^T/V stay 64-scaled in bf16 (the descale folds into
    the exp scale (64*64) and the final 1/den reciprocal).
  - Softmax: no max-subtraction needed (scores are O(1)); P^T =
    exp(S^T*scale)*mask^T on DVE (bf16, feeds U) and in parallel on
    GpSimd (fp8, feeds the denominator). The denominator is ONE fp8
    DoubleRow ones-matmul per pair of T-blocks (4x cheaper than bf16).
  - Rows with all-false mask give P = 0 exactly on both copies, so
    U = 0 and den = 0 -> max(den, tiny) keeps the reference's
    wipe-masking semantics.

Matmul inputs are fp8/bf16; PSUM accumulation, softmax denominators
and reciprocal stay fp32.
"""

import sys

if "/opt/trn_rl_repo" not in sys.path:
    sys.path.insert(0, "/opt/trn_rl_repo")

import numpy as np
import ml_dtypes
from contextlib import ExitStack

B, TQ, TKV, D, H = 4, 512, 4096, 1024, 8
HD = D // H            # 128 head dim
NCORES = 8
GH = H // 2            # heads per core = 4
GD = GH * HD           # 512 cols per head-group
P = 128
KC = D // P            # 8 contraction chunks
NTB = TKV // P         # 32 T-blocks
NTC = TKV // 512       # 8 T-chunks (DMA granularity)
NOB = GD // P          # 4 output blocks per core (own col half)
WS = 64.0              # host weight pre-scale (keeps fp8 lo normal)
SCALE = float(1.0 / np.sqrt(HD))
ESCALE = SCALE / (WS * WS)   # exp scale compensating 64-scaled Q and K

_CACHED_NC = None


def _build_nc():
    from concourse import mybir, bacc
    from concourse.tile import TileContext

    bf = mybir.dt.bfloat16
    f8 = mybir.dt.float8e4
    f32 = mybir.dt.float32
    AF = mybir.ActivationFunctionType
    OP = mybir.AluOpType
    DR = mybir.MatmulPerfMode.DoubleRow

    nc = bacc.Bacc("TRN2", target_bir_lowering=False, debug=False,
                   num_devices=NCORES)

    # All inputs are pre-tiled on the host into partition-major layouts
    # so every DMA is 128 contiguous multi-KB descriptors. fp8 tensors
    # carry (hi, lo) split pairs in their '2' axis.
    xq8 = nc.dram_tensor("xq8", [P, KC, 2, TQ], f8, kind="ExternalInput")
    xkv8 = nc.dram_tensor("xkv8", [P, NTC, KC, 2, 512], f8,
                          kind="ExternalInput")
    mask8 = nc.dram_tensor("mask8", [P, NTB, TQ], f8, kind="ExternalInput")
    Wq8 = nc.dram_tensor("Wq8", [P, KC, 2, GD], f8, kind="ExternalInput")
    Wk8 = nc.dram_tensor("Wk8", [P, KC, 2, GD], f8, kind="ExternalInput")
    Wv8 = nc.dram_tensor("Wv8", [P, KC, 2, GD], f8, kind="ExternalInput")
    Wo = nc.dram_tensor("Wo", [P, H, GD], bf, kind="ExternalInput")
    bq64 = nc.dram_tensor("bq64", [GD], f32, kind="ExternalInput")
    bk64 = nc.dram_tensor("bk64", [GD], f32, kind="ExternalInput")
    bv64 = nc.dram_tensor("bv64", [GD], f32, kind="ExternalInput")
    bo = nc.dram_tensor("bo", [GD], f32, kind="ExternalInput")
    out = nc.dram_tensor("out", [P, NOB, TQ], bf, kind="ExternalOutput")

    with TileContext(nc) as tc:
        with ExitStack() as ctx:
            persist = ctx.enter_context(tc.tile_pool(name="persist", bufs=1))
            kvchunk = ctx.enter_context(tc.tile_pool(name="kvchunk", bufs=3))
            work = ctx.enter_context(tc.tile_pool(name="work", bufs=3))
            outp = ctx.enter_context(tc.tile_pool(name="outp", bufs=1))
            ppool = ctx.enter_context(
                tc.tile_pool(name="ppool", bufs=2, space="PSUM"))
            upool = ctx.enter_context(
                tc.tile_pool(name="upool", bufs=2, space="PSUM"))
            dpool = ctx.enter_context(
                tc.tile_pool(name="dpool", bufs=2, space="PSUM"))
            dram = ctx.enter_context(
                tc.tile_pool(name="dram", bufs=1, space="DRAM"))

            # ---- constants / weights / biases -------------------------
            # DMA emission order matters for time-to-first-matmul: Wq+xq
            # first so the Q projection starts ASAP, then Wk/Wv + kv
            # chunks; mask/Wo are only needed later.
            wq_sb = persist.tile([P, KC, 2, GD], f8)
            xq_sb = persist.tile([P, KC, 2, TQ], f8)
            nc.sync.dma_start(wq_sb[:, 0:1, :, :], Wq8.ap()[:, 0:1, :, :])
            nc.sync.dma_start(xq_sb[:, 0:1, :, :], xq8.ap()[:, 0:1, :, :])
            nc.sync.dma_start(wq_sb[:, 1:, :, :], Wq8.ap()[:, 1:, :, :])
            nc.sync.dma_start(xq_sb[:, 1:, :, :], xq8.ap()[:, 1:, :, :])

            bq_sb = persist.tile([P, GH], f32)
            bk_sb = persist.tile([P, GH], f32)
            nc.sync.dma_start(bq_sb[:], bq64.ap().rearrange("(h p) -> p h", p=P))
            nc.sync.dma_start(bk_sb[:], bk64.ap().rearrange("(h p) -> p h", p=P))
            bv_row = persist.tile([1, GD], f32)
            nc.sync.dma_start(bv_row[:], bv64.ap().unsqueeze(0))
            bv_rep = persist.tile([P, GD], f32)
            nc.gpsimd.partition_broadcast(bv_rep[:], bv_row[:])

            ones8 = persist.tile([P, 2, P], f8)
            nc.vector.memset(ones8[:], 1.0)

            wk_sb = persist.tile([P, KC, 2, GD], f8)
            wv_sb = persist.tile([P, KC, 2, GD], f8)
            kv_tiles = {}

            def load_kv_chunk(tcknk):
                t = kvchunk.tile([P, KC, 2, 512], f8, name="xkv_t", tag="xkv")
                nc.sync.dma_start(t[:], xkv8.ap()[:, tcknk, :, :, :])
                kv_tiles[tcknk] = t

            nc.sync.dma_start(wk_sb[:], Wk8.ap())
            load_kv_chunk(0)
            nc.sync.dma_start(wv_sb[:], Wv8.ap())
            load_kv_chunk(1)

            def proj_3term(ps, w8, x8, nfree):
                """psum[.,nfree] = (W^T x) via 3-term fp8 DoubleRow.

                w8: [P, KC, 2, M-stride view] slot pairs (hi, lo)
                x8: [P, KC, 2, nfree] slot pairs (hi, lo)
                M1(kc): (Whi,xhi)+(Wlo,xhi): rhs = xhi broadcast pair
                M2(i):  (Whi_2i,xlo_2i)+(Whi_2i+1,xlo_2i+1)
                """
                for kc in range(KC):
                    rhs = x8[:, kc, 0:1, :].to_broadcast([P, 2, nfree])
                    nc.tensor.matmul(ps[:], w8[:, kc, :, :], rhs,
                                     start=(kc == 0), stop=False,
                                     perf_mode=DR)
                for i in range(KC // 2):
                    lhsT = w8[:, 2 * i:2 * i + 2, 0, :]
                    rhs = x8[:, 2 * i:2 * i + 2, 1, :]
                    nc.tensor.matmul(ps[:], lhsT, rhs,
                                     start=False, stop=(i == KC // 2 - 1),
                                     perf_mode=DR)

            # ---- Q^T = 64*(Wq_g^T x_q^T + bq) -------------------------
            qt_sb = persist.tile([P, GH, TQ], bf)
            for db in range(GH):
                ps = ppool.tile([P, 2, TQ], f32, name="proj_ps",
                                tag="big")[:, 0, :]
                proj_3term(ps, wq_sb[:, :, :, db * P:(db + 1) * P],
                           xq_sb, TQ)
                nc.vector.tensor_tensor(
                    qt_sb[:, db, :], ps[:],
                    bq_sb[:, db:db + 1].to_broadcast([P, TQ]), OP.add)

            # ---- K^T and V over T-chunks (64-scaled) ------------------
            kt_sb = persist.tile([P, GH, TKV], bf)
            v_sb = persist.tile([P, NTB, GD], bf)
            mask_sb = persist.tile([P, NTB, TQ], f8)
            bo_sb = persist.tile([P, NOB], f32)
            wo_sb = persist.tile([P, H, GD], bf)
            for tcknk in range(NTC):
                if tcknk + 2 < NTC:
                    load_kv_chunk(tcknk + 2)
                xkv_t = kv_tiles.pop(tcknk)
                if tcknk == 1:
                    # queue the bulk "later-phase" loads behind chunks 0-1
                    nc.sync.dma_start(mask_sb[:], mask8.ap())
                    nc.sync.dma_start(wo_sb[:], Wo.ap())
                    nc.sync.dma_start(
                        bo_sb[:], bo.ap().rearrange("(ob p) -> p ob", p=P))
                for db in range(GH):
                    ps = ppool.tile([P, 2, 512], f32, name="proj_ps",
                                    tag="big")[:, 0, :]
                    proj_3term(ps, wk_sb[:, :, :, db * P:(db + 1) * P],
                               xkv_t, 512)
                    nc.vector.tensor_tensor(
                        kt_sb[:, db, tcknk * 512:(tcknk + 1) * 512], ps[:],
                        bk_sb[:, db:db + 1].to_broadcast([P, 512]), OP.add)
                for tb in range(4):
                    ps = ppool.tile([P, 2, 512], f32, name="proj_ps",
                                    tag="big")[:, 0, :]
                    # V = x^T W: lhsT = x pairs, rhs = W
                    for kc in range(KC):
                        rhs = wv_sb[:, kc, 0:1, :].to_broadcast([P, 2, GD])
                        nc.tensor.matmul(
                            ps[:], xkv_t[:, kc, :, tb * P:(tb + 1) * P], rhs,
                            start=(kc == 0), stop=False, perf_mode=DR)
                    for i in range(KC // 2):
                        nc.tensor.matmul(
                            ps[:], xkv_t[:, 2 * i:2 * i + 2, 0,
                                         tb * P:(tb + 1) * P],
                            wv_sb[:, 2 * i:2 * i + 2, 1, :],
                            start=False, stop=(i == KC // 2 - 1),
                            perf_mode=DR)
                    nc.vector.tensor_tensor(
                        v_sb[:, tcknk * 4 + tb, :], ps[:], bv_rep[:], OP.add)

            # ---- attention, flattened double-step loop ----------------
            # Two T-blocks per step: two S-matmuls fill the two banks of
            # one [P, 2, TQ] psum tile, then ONE wide exp, one wide
            # mask-mult on DVE (bf16 -> U) and one on GpSimd (fp8 -> den).
            ut_sb = persist.tile([P, GH, TQ], bf)
            u_all = persist.tile([P, 2, GH, TQ], bf)
            cc_in = [dram.tile([P, TQ], bf, name=f"cc_in{h}")
                     for h in range(GH)]
            cc_out = [dram.tile([2, P, TQ], bf, name=f"cc_out{h}")
                      for h in range(GH)]
            RG = [[0, 4], [1, 5], [2, 6], [3, 7]]

            NDS = GH * NTB // 2
            s_tiles = {}
            u_tiles = [None] * GH
            den_tiles = [None] * GH
            SPRE = 2  # double-step prefetch depth

            def s2_mm(ds):
                t2 = ppool.tile([P, 2, TQ], f32, name="s2_ps", tag="big")
                for k in range(2):
                    h, j = divmod(ds * 2 + k, NTB)
                    nc.tensor.matmul(t2[:, k, :],
                                     kt_sb[:, h, j * P:(j + 1) * P],
                                     qt_sb[:, h, :], start=True, stop=True)
                return t2

            for pre in range(SPRE):
                s_tiles[pre] = s2_mm(pre)
            for ds in range(NDS):
                h, j0 = divmod(ds * 2, NTB)
                if j0 == 0:
                    u_tiles[h] = upool.tile([P, TQ], f32, name="u_ps",
                                            tag="u_ps")
                    den_tiles[h] = dpool.tile([P, TQ], f32, name="den_ps",
                                              tag="den_ps")
                t2 = s_tiles.pop(ds)
                praw = work.tile([P, 2, TQ], bf, tag="praw", bufs=2)
                nc.scalar.activation(praw[:], t2[:], AF.Exp, scale=ESCALE)
                p_t = work.tile([P, 2, TQ], bf, tag="p_t", bufs=2)
                nc.vector.tensor_tensor(p_t[:], praw[:],
                                        mask_sb[:, j0:j0 + 2, :], OP.mult)
                p8 = work.tile([P, 2, TQ], f8, tag="p8", bufs=2)
                nc.gpsimd.tensor_tensor(p8[:], praw[:],
                                        mask_sb[:, j0:j0 + 2, :], OP.mult)
                if ds + SPRE < NDS:
                    s_tiles[ds + SPRE] = s2_mm(ds + SPRE)
                for k in range(2):
                    j = j0 + k
                    nc.tensor.matmul(u_tiles[h][:],
                                     v_sb[:, j, h * P:(h + 1) * P],
                                     p_t[:, k, :],
                                     start=(j == 0), stop=(j == NTB - 1))
                nc.tensor.matmul(den_tiles[h][:], ones8[:], p8[:],
                                 start=(j0 == 0), stop=(j0 + 2 == NTB),
                                 perf_mode=DR)
                if j0 + 2 == NTB:
                    den_sf = work.tile([1, TQ], f32, tag="den_sf")
                    nc.vector.tensor_scalar(den_sf[:], den_tiles[h][0:1, :],
                                            1e-30, WS, OP.max, OP.mult)
                    recip = work.tile([1, TQ], f32, tag="recip")
                    nc.vector.reciprocal(recip[:], den_sf[:])
                    recip_rep = work.tile([P, TQ], f32, tag="recip_rep")
                    nc.gpsimd.partition_broadcast(recip_rep[:], recip[:])
                    nc.vector.tensor_tensor(ut_sb[:, h, :], u_tiles[h][:],
                                            recip_rep[:], OP.mult)
                    # pair-exchange this head's U while attention continues
                    nc.sync.dma_start(cc_in[h][:], ut_sb[:, h, :])
                    nc.gpsimd.collective_compute(
                        "AllGather", OP.bypass, replica_groups=RG,
                        ins=[cc_in[h].opt()], outs=[cc_out[h].opt()])
                    nc.sync.dma_start(
                        u_all[:, :, h, :],
                        cc_out[h][:].rearrange("r p t -> p r t"))

            # ---- out cols [g*512,(g+1)*512) = Wo_own^T U_all (+bo) ----
            o_sb = outp.tile([P, NOB, TQ], bf)
            for ob in range(NOB):
                ps = ppool.tile([P, 2, TQ], f32, name="proj_ps",
                                tag="big")[:, 0, :]
                for hh in range(H):
                    r, lh = divmod(hh, GH)
                    nc.tensor.matmul(ps[:],
                                     wo_sb[:, hh, ob * P:(ob + 1) * P],
                                     u_all[:, r, lh, :],
                                     start=(hh == 0), stop=(hh == H - 1))
                nc.vector.tensor_tensor(
                    o_sb[:, ob, :], ps[:],
                    bo_sb[:, ob:ob + 1].to_broadcast([P, TQ]), OP.add)
            nc.sync.dma_start(out.ap(), o_sb[:])

    nc.finalize()
    return nc


def _split8(a, scale=1.0):
    """f32 array -> (hi, lo) float8_e4m3 pair with hi+lo ~= scale*a."""
    f8 = ml_dtypes.float8_e4m3
    a = np.asarray(a, np.float32) * np.float32(scale)
    hi = a.astype(f8)
    lo = (a - hi.astype(np.float32)).astype(f8)
    return hi, lo


def _ptile(a2d, inner):
    """[R, C] row-major -> [P, R//P, C] partition-major, contiguous."""
    r, c = a2d.shape
    return np.ascontiguousarray(
        a2d.reshape(r // P, P, c).transpose(1, 0, 2)).astype(inner)


def _w_split(w, sl_cols=None, sl_rows=None):
    """Weight shard -> [P, KC, 2, cols] fp8 (hi, lo), 64-scaled."""
    f8 = ml_dtypes.float8_e4m3
    w = np.ascontiguousarray(w if sl_cols is None else w[:, sl_cols])
    hi, lo = _split8(w, WS)
    # [D, C] -> [P, KC, C] each, stack (hi, lo) on axis 2
    hp = _ptile(hi.astype(np.float32), np.float32)
    lp = _ptile(lo.astype(np.float32), np.float32)
    return np.ascontiguousarray(
        np.stack([hp, lp], axis=2)).astype(f8)


def _shard_inputs(inputs_q, inputs_kv, attention_mask, Wq, bq, Wk, bk, Wv, bv,
                  Wo, bo):
    f8 = ml_dtypes.float8_e4m3
    bf16 = ml_dtypes.bfloat16
    f32 = np.float32

    def x_split(x2d):
        """x^T [D, T] -> [P, KC, 2, T] fp8 (hi, lo)."""
        hi, lo = _split8(x2d)
        hp = _ptile(hi.astype(np.float32), np.float32)
        lp = _ptile(lo.astype(np.float32), np.float32)
        return np.ascontiguousarray(np.stack([hp, lp], axis=2)).astype(f8)

    xq8 = [x_split(inputs_q[b].T) for b in range(B)]         # [P,KC,2,TQ]
    # x_kv^T -> [P, KC, 2, TKV] -> chunk to [P, NTC, KC, 2, 512]
    xkv8 = []
    for b in range(B):
        t = x_split(inputs_kv[b].T)                          # [P,KC,2,TKV]
        t = t.reshape(P, KC, 2, NTC, 512).transpose(0, 3, 1, 2, 4)
        xkv8.append(np.ascontiguousarray(t))
    mask8 = [_ptile(attention_mask[b].T.astype(np.float32), f8)
             for b in range(B)]                              # [P,NTB,TQ]

    in_maps = []
    for c in range(NCORES):
        b, g = c % B, c // B  # pair = (b, b+4)
        sl = slice(g * GD, (g + 1) * GD)
        in_maps.append({
            "xq8": xq8[b],
            "xkv8": xkv8[b],
            "mask8": mask8[b],
            "Wq8": _w_split(Wq, sl_cols=sl),
            "Wk8": _w_split(Wk, sl_cols=sl),
            "Wv8": _w_split(Wv, sl_cols=sl),
            # all head rows x own col half, [P, H, GD] bf16
            "Wo": _ptile(np.ascontiguousarray(Wo[:, sl]), bf16),
            "bq64": np.ascontiguousarray(bq[sl]).astype(f32) * WS,
            "bk64": np.ascontiguousarray(bk[sl]).astype(f32) * WS,
            "bv64": np.ascontiguousarray(bv[sl]).astype(f32) * WS,
            "bo": np.ascontiguousarray(bo[sl]).astype(f32),
        })
    return in_maps


def kernel(_trace=False, **inputs):
    global _CACHED_NC
    from concourse import bass_utils

    arrs = {k: np.asarray(v) for k, v in inputs.items()}
    in_maps = _shard_inputs(**arrs)

    if _CACHED_NC is None:
        _CACHED_NC = _build_nc()

    res = bass_utils.run_bass_kernel_spmd(
        _CACHED_NC, in_maps, core_ids=list(range(NCORES)), trace=_trace)

    full = np.empty((B, TQ, D), np.float32)
    for c in range(NCORES):
        b, g = c % B, c // B
        o = res.results[c]["out"]  # [P, NOB, TQ] bf16, o-col = ob*128+p
        full[b][:, g * GD:(g + 1) * GD] = (
            o.transpose(2, 1, 0).reshape(TQ, GD).astype(np.float32))
    if _trace:
        return full, res
    return full


# revision 8
# speedup vs baseline: 1.2728x; 1.2728x over previous
"""Distributed attention kernel for one TRN2 chip (8 NeuronCores).

Problem: multi-head cross-attention
  B=4, TQ=512, TKV=4096, D=1024, H=8 heads (head_dim=128)

Sharding (data-parallel x tensor-parallel, per the hint):
  core c in 0..7 -> (batch b = c % 4, head-group g = c // 4)
  Each core computes heads [4g, 4g+4) for its batch (Wq/Wk/Wv column
  shards). Per-head U results are pair-exchanged (c <-> c+4) with an
  AllGather DURING the attention phase, so every core ends with all 8
  heads' U and computes its own 512-column slice of the output
  projection locally - no serialized collective tail.

Device layout (per core; everything transposed so no on-device
transposes are needed - the host passes x^T and mask^T):
  Q^T[dh, t]  = Wq_g^T x_q^T          (4 head-blocks x 8 k-chunks)
  K^T[dh, T]  = Wk_g^T x_kv^T
  V[T, dh]    = x_kv Wv_g             (from x_kv^T chunks as lhsT)
  S^T[T, t]   = K^T_h(block)^T Q^T_h  per head, 32 T-blocks
  praw        = exp(S^T/sqrt(128))    (no max-subtraction needed:
                scores are O(1) so exp cannot overflow/underflow)
  P^T         = praw * mask^T  (bf16, feeds U)   [DVE]
  P8          = praw * mask^T  (fp8, feeds den)  [DVE, fp8 out]
  U^T[dh, t] += V_h(block)^T P^T      accumulated over T-blocks in PSUM
  den        += ones8^T P8            fp8 DoubleRow: ONE matmul per
                                      PAIR of T-blocks (2x cheaper)
  U^T *= 1/max(den, tiny)             (rows with all-false mask give
                U = 0 exactly, so they stay 0 like the reference wipe)
  per head: pair AllGather of U^T, overlapped with attention
  out^T[o_own, t] = Wo_own^T U_all^T (+ bo slice), direct DMA out.

Matmul inputs are bf16 (PE 4x faster than fp32); PSUM accumulation,
softmax denominators and reciprocal stay fp32.
"""

import sys

if "/opt/trn_rl_repo" not in sys.path:
    sys.path.insert(0, "/opt/trn_rl_repo")

import numpy as np
import ml_dtypes
from contextlib import ExitStack

B, TQ, TKV, D, H = 4, 512, 4096, 1024, 8
HD = D // H            # 128 head dim
NCORES = 8
GH = H // 2            # heads per core = 4
GD = GH * HD           # 512 cols per head-group
P = 128
KC = D // P            # 8 contraction chunks
NTB = TKV // P         # 32 T-blocks
NTC = TKV // 512       # 8 T-chunks (DMA granularity)
NOB = GD // P          # 4 output blocks per core (own col half)
SCALE = float(1.0 / np.sqrt(HD))

_CACHED_NC = None


def _build_nc():
    from concourse import mybir, bacc
    from concourse.tile import TileContext

    bf = mybir.dt.bfloat16
    f8 = mybir.dt.float8e4
    f32 = mybir.dt.float32
    AF = mybir.ActivationFunctionType
    OP = mybir.AluOpType
    DR = mybir.MatmulPerfMode.DoubleRow

    nc = bacc.Bacc("TRN2", target_bir_lowering=False, debug=False,
                   num_devices=NCORES)

    # All inputs are pre-tiled on the host into partition-major layouts
    # so every DMA is 128 contiguous multi-KB descriptors.
    xqT = nc.dram_tensor("xqT", [P, KC, TQ], bf, kind="ExternalInput")
    xkvT = nc.dram_tensor("xkvT", [P, NTC, KC, 512], bf, kind="ExternalInput")
    maskT = nc.dram_tensor("maskT", [P, NTB, TQ], bf, kind="ExternalInput")
    Wq = nc.dram_tensor("Wq", [P, KC, GD], bf, kind="ExternalInput")
    Wk = nc.dram_tensor("Wk", [P, KC, GD], bf, kind="ExternalInput")
    Wv = nc.dram_tensor("Wv", [P, KC, GD], bf, kind="ExternalInput")
    Wo = nc.dram_tensor("Wo", [P, H, GD], bf, kind="ExternalInput")
    bq = nc.dram_tensor("bq", [GD], f32, kind="ExternalInput")
    bk = nc.dram_tensor("bk", [GD], f32, kind="ExternalInput")
    bv = nc.dram_tensor("bv", [GD], f32, kind="ExternalInput")
    bo = nc.dram_tensor("bo", [GD], f32, kind="ExternalInput")
    out = nc.dram_tensor("out", [P, NOB, TQ], bf, kind="ExternalOutput")

    with TileContext(nc) as tc:
        with ExitStack() as ctx:
            persist = ctx.enter_context(tc.tile_pool(name="persist", bufs=1))
            kvchunk = ctx.enter_context(tc.tile_pool(name="kvchunk", bufs=3))
            work = ctx.enter_context(tc.tile_pool(name="work", bufs=3))
            outp = ctx.enter_context(tc.tile_pool(name="outp", bufs=1))
            # One pool of double-bank [P, 2, TQ] psum tiles serves the
            # projections (using one half) and the attention S-tiles.
            ppool = ctx.enter_context(
                tc.tile_pool(name="ppool", bufs=2, space="PSUM"))
            upool = ctx.enter_context(
                tc.tile_pool(name="upool", bufs=2, space="PSUM"))
            dpool = ctx.enter_context(
                tc.tile_pool(name="dpool", bufs=2, space="PSUM"))
            dram = ctx.enter_context(
                tc.tile_pool(name="dram", bufs=1, space="DRAM"))

            # ---- constants / weights / biases -------------------------
            # DMA emission order matters for time-to-first-matmul: Wq+xq
            # first so the Q projection starts ~6us in, then Wk/Wv, then
            # the kv chunks; mask/Wo are only needed later.
            wq_sb = persist.tile([P, KC, GD], bf)
            xq_sb = persist.tile([P, KC, TQ], bf)
            nc.sync.dma_start(wq_sb[:, 0:1, :], Wq.ap()[:, 0:1, :])
            nc.sync.dma_start(xq_sb[:, 0:1, :], xqT.ap()[:, 0:1, :])
            nc.sync.dma_start(wq_sb[:, 1:, :], Wq.ap()[:, 1:, :])
            nc.sync.dma_start(xq_sb[:, 1:, :], xqT.ap()[:, 1:, :])

            bq_sb = persist.tile([P, GH], f32)
            bk_sb = persist.tile([P, GH], f32)
            nc.sync.dma_start(bq_sb[:], bq.ap().rearrange("(h p) -> p h", p=P))
            nc.sync.dma_start(bk_sb[:], bk.ap().rearrange("(h p) -> p h", p=P))
            bv_row = persist.tile([1, GD], f32)
            nc.sync.dma_start(bv_row[:], bv.ap().unsqueeze(0))
            bv_rep = persist.tile([P, GD], f32)
            nc.gpsimd.partition_broadcast(bv_rep[:], bv_row[:])

            ones8 = persist.tile([P, 2, P], f8)
            nc.vector.memset(ones8[:], 1.0)

            wk_sb = persist.tile([P, KC, GD], bf)
            wv_sb = persist.tile([P, KC, GD], bf)
            kv_tiles = {}

            def load_kv_chunk(tcknk):
                t = kvchunk.tile([P, KC, 512], bf, name="xkv_t", tag="xkv")
                nc.sync.dma_start(t[:], xkvT.ap()[:, tcknk, :, :])
                kv_tiles[tcknk] = t

            nc.sync.dma_start(wk_sb[:], Wk.ap())
            load_kv_chunk(0)
            nc.sync.dma_start(wv_sb[:], Wv.ap())
            load_kv_chunk(1)

            # ---- Q^T = Wq_g^T x_q^T  (+bq) ----------------------------
            qt_sb = persist.tile([P, GH, TQ], bf)
            for db in range(GH):
                ps = ppool.tile([P, 2, TQ], f32, name="proj_ps",
                                tag="big")[:, 0, :]
                for kc in range(KC):
                    nc.tensor.matmul(ps[:], wq_sb[:, kc, db * P:(db + 1) * P],
                                     xq_sb[:, kc, :],
                                     start=(kc == 0), stop=(kc == KC - 1))
                nc.vector.tensor_tensor(
                    qt_sb[:, db, :], ps[:],
                    bq_sb[:, db:db + 1].to_broadcast([P, TQ]), OP.add)

            # ---- K^T and V over T-chunks ------------------------------
            kt_sb = persist.tile([P, GH, TKV], bf)
            v_sb = persist.tile([P, NTB, GD], bf)
            mask_sb = persist.tile([P, NTB, TQ], bf)
            bo_sb = persist.tile([P, NOB], f32)
            wo_sb = persist.tile([P, H, GD], bf)
            for tcknk in range(NTC):
                if tcknk + 2 < NTC:
                    load_kv_chunk(tcknk + 2)
                xkv_t = kv_tiles.pop(tcknk)
                if tcknk == 1:
                    # queue the bulk "later-phase" loads behind chunks 0-1
                    nc.sync.dma_start(mask_sb[:], maskT.ap())
                    nc.sync.dma_start(wo_sb[:], Wo.ap())
                    nc.sync.dma_start(
                        bo_sb[:], bo.ap().rearrange("(ob p) -> p ob", p=P))
                for db in range(GH):
                    ps = ppool.tile([P, 2, 512], f32, name="proj_ps",
                                    tag="big")[:, 0, :]
                    for kc in range(KC):
                        nc.tensor.matmul(ps[:], wk_sb[:, kc, db * P:(db + 1) * P],
                                         xkv_t[:, kc, :],
                                         start=(kc == 0), stop=(kc == KC - 1))
                    nc.vector.tensor_tensor(
                        kt_sb[:, db, tcknk * 512:(tcknk + 1) * 512], ps[:],
                        bk_sb[:, db:db + 1].to_broadcast([P, 512]), OP.add)
                for tb in range(4):
                    ps = ppool.tile([P, 2, 512], f32, name="proj_ps",
                                    tag="big")[:, 0, :]
                    for kc in range(KC):
                        nc.tensor.matmul(ps[:],
                                         xkv_t[:, kc, tb * P:(tb + 1) * P],
                                         wv_sb[:, kc, :],
                                         start=(kc == 0), stop=(kc == KC - 1))
                    nc.vector.tensor_tensor(
                        v_sb[:, tcknk * 4 + tb, :], ps[:], bv_rep[:], OP.add)

            # ---- attention, flattened double-step loop ----------------
            # Two T-blocks per step: two S-matmuls fill the two banks of
            # one [P, 2, TQ] psum tile, then ONE wide exp (ACT per-op
            # overhead amortized below the PE pace), one wide mask-mult
            # (bf16, feeds U) and one wide mask-mult to fp8 (feeds den).
            ut_sb = persist.tile([P, GH, TQ], bf)
            u_all = persist.tile([P, 2, GH, TQ], bf)
            cc_in = [dram.tile([P, TQ], bf, name=f"cc_in{h}")
                     for h in range(GH)]
            cc_out = [dram.tile([2, P, TQ], bf, name=f"cc_out{h}")
                      for h in range(GH)]
            RG = [[0, 4], [1, 5], [2, 6], [3, 7]]

            NDS = GH * NTB // 2
            s_tiles = {}
            u_tiles = [None] * GH
            den_tiles = [None] * GH
            SPRE = 2  # double-step prefetch depth

            def s2_mm(ds):
                t2 = ppool.tile([P, 2, TQ], f32, name="s2_ps", tag="big")
                for k in range(2):
                    h, j = divmod(ds * 2 + k, NTB)
                    nc.tensor.matmul(t2[:, k, :],
                                     kt_sb[:, h, j * P:(j + 1) * P],
                                     qt_sb[:, h, :], start=True, stop=True)
                return t2

            for pre in range(SPRE):
                s_tiles[pre] = s2_mm(pre)
            for ds in range(NDS):
                h, j0 = divmod(ds * 2, NTB)
                if j0 == 0:
                    u_tiles[h] = upool.tile([P, TQ], f32, name="u_ps",
                                            tag="u_ps")
                    den_tiles[h] = dpool.tile([P, TQ], f32, name="den_ps",
                                              tag="den_ps")
                t2 = s_tiles.pop(ds)
                praw = work.tile([P, 2, TQ], bf, tag="praw", bufs=2)
                nc.scalar.activation(praw[:], t2[:], AF.Exp, scale=SCALE)
                p_t = work.tile([P, 2, TQ], bf, tag="p_t", bufs=2)
                nc.vector.tensor_tensor(p_t[:], praw[:],
                                        mask_sb[:, j0:j0 + 2, :], OP.mult)
                p8 = work.tile([P, 2, TQ], f8, tag="p8", bufs=2)
                nc.vector.tensor_tensor(p8[:], praw[:],
                                        mask_sb[:, j0:j0 + 2, :], OP.mult)
                if ds + SPRE < NDS:
                    s_tiles[ds + SPRE] = s2_mm(ds + SPRE)
                for k in range(2):
                    j = j0 + k
                    nc.tensor.matmul(u_tiles[h][:],
                                     v_sb[:, j, h * P:(h + 1) * P],
                                     p_t[:, k, :],
                                     start=(j == 0), stop=(j == NTB - 1))
                nc.tensor.matmul(den_tiles[h][:], ones8[:], p8[:],
                                 start=(j0 == 0), stop=(j0 + 2 == NTB),
                                 perf_mode=DR)
                if j0 + 2 == NTB:
                    den_sf = work.tile([1, TQ], f32, tag="den_sf", bufs=2)
                    nc.vector.tensor_scalar(den_sf[:], den_tiles[h][0:1, :],
                                            1e-30, None, OP.max)
                    recip = work.tile([1, TQ], f32, tag="recip", bufs=2)
                    nc.vector.reciprocal(recip[:], den_sf[:])
                    recip_rep = work.tile([P, TQ], f32, tag="recip_rep", bufs=2)
                    nc.gpsimd.partition_broadcast(recip_rep[:], recip[:])
                    nc.vector.tensor_tensor(ut_sb[:, h, :], u_tiles[h][:],
                                            recip_rep[:], OP.mult)
                    # pair-exchange this head's U while attention continues
                    nc.sync.dma_start(cc_in[h][:], ut_sb[:, h, :])
                    nc.gpsimd.collective_compute(
                        "AllGather", OP.bypass, replica_groups=RG,
                        ins=[cc_in[h].opt()], outs=[cc_out[h].opt()])
                    nc.sync.dma_start(
                        u_all[:, :, h, :],
                        cc_out[h][:].rearrange("r p t -> p r t"))

            # ---- out cols [g*512,(g+1)*512) = Wo_own^T U_all (+bo) ----
            o_sb = outp.tile([P, NOB, TQ], bf)
            for ob in range(NOB):
                ps = ppool.tile([P, 2, TQ], f32, name="proj_ps",
                                tag="big")[:, 0, :]
                for hh in range(H):
                    r, lh = divmod(hh, GH)
                    nc.tensor.matmul(ps[:],
                                     wo_sb[:, hh, ob * P:(ob + 1) * P],
                                     u_all[:, r, lh, :],
                                     start=(hh == 0), stop=(hh == H - 1))
                nc.vector.tensor_tensor(
                    o_sb[:, ob, :], ps[:],
                    bo_sb[:, ob:ob + 1].to_broadcast([P, TQ]), OP.add)
            nc.sync.dma_start(out.ap(), o_sb[:])

    nc.finalize()
    return nc


def _ptile(a2d, inner):
    """[R, C] row-major -> [P, R//P, C] partition-major, contiguous."""
    r, c = a2d.shape
    return np.ascontiguousarray(
        a2d.reshape(r // P, P, c).transpose(1, 0, 2)).astype(inner)


def _shard_inputs(inputs_q, inputs_kv, attention_mask, Wq, bq, Wk, bk, Wv, bv,
                  Wo, bo):
    bf16 = ml_dtypes.bfloat16
    f32 = np.float32

    xqT = [_ptile(inputs_q[b].T, bf16) for b in range(B)]         # [P,KC,TQ]
    xkvT = [_ptile(inputs_kv[b].T, bf16)                          # [P,NTC,KC,512]
            .reshape(P, KC, NTC, 512).transpose(0, 2, 1, 3).copy()
            for b in range(B)]
    maskT = [_ptile(attention_mask[b].T.astype(np.float32), bf16)  # [P,NTB,TQ]
             for b in range(B)]
    in_maps = []
    for c in range(NCORES):
        b, g = c % B, c // B  # pair = (b, b+4)
        sl = slice(g * GD, (g + 1) * GD)
        in_maps.append({
            "xqT": xqT[b],
            "xkvT": xkvT[b],
            "maskT": maskT[b],
            "Wq": _ptile(np.ascontiguousarray(Wq[:, sl]), bf16),
            "Wk": _ptile(np.ascontiguousarray(Wk[:, sl]), bf16),
            "Wv": _ptile(np.ascontiguousarray(Wv[:, sl]), bf16),
            # all head rows x own col half, [P, H, GD] bf16
            "Wo": _ptile(np.ascontiguousarray(Wo[:, sl]), bf16),
            "bq": np.ascontiguousarray(bq[sl]).astype(f32),
            "bk": np.ascontiguousarray(bk[sl]).astype(f32),
            "bv": np.ascontiguousarray(bv[sl]).astype(f32),
            "bo": np.ascontiguousarray(bo[sl]).astype(f32),
        })
    return in_maps


def kernel(_trace=False, **inputs):
    global _CACHED_NC
    from concourse import bass_utils

    arrs = {k: np.asarray(v) for k, v in inputs.items()}
    in_maps = _shard_inputs(**arrs)

    if _CACHED_NC is None:
        _CACHED_NC = _build_nc()

    res = bass_utils.run_bass_kernel_spmd(
        _CACHED_NC, in_maps, core_ids=list(range(NCORES)), trace=_trace)

    full = np.empty((B, TQ, D), np.float32)
    for c in range(NCORES):
        b, g = c % B, c // B
        o = res.results[c]["out"]  # [P, NOB, TQ] bf16, o-col = ob*128+p
        full[b][:, g * GD:(g + 1) * GD] = (
            o.transpose(2, 1, 0).reshape(TQ, GD).astype(np.float32))
    if _trace:
        return full, res
    return full


# revision 12
# speedup vs baseline: 1.5293x; 1.2015x over previous
"""Distributed attention kernel for one TRN2 chip (8 NeuronCores).

Problem: multi-head cross-attention
  B=4, TQ=512, TKV=4096, D=1024, H=8 heads (head_dim=128)

Sharding (data-parallel x tensor-parallel, per the hint):
  core c in 0..7 -> (batch b = c % 4, head-group g = c // 4)
  Each core computes heads [4g, 4g+4) for its batch (Wq/Wk/Wv column
  shards). Per-head U results are pair-exchanged (c <-> c+4) with an
  AllGather DURING the attention phase, so every core ends with all 8
  heads' U and computes its own 512-column slice of the output
  projection locally - no serialized collective tail.

Device layout (per core; everything transposed so no on-device
transposes are needed - the host passes x^T and mask^T):
  Q^T[dh, t]  = Wq_g^T x_q^T          (4 head-blocks x 8 k-chunks)
  K^T[dh, T]  = Wk_g^T x_kv^T
  V[T, dh]    = x_kv Wv_g             (from x_kv^T chunks as lhsT)
  S^T[T, t]   = K^T_h(block)^T Q^T_h  per head, 32 T-blocks
  praw        = exp(S^T/sqrt(128))    (no max-subtraction needed:
                scores are O(1) so exp cannot overflow/underflow)
  P^T         = praw * mask^T  (bf16, feeds U)   [DVE]
  P8          = praw * mask^T  (fp8, feeds den)  [DVE, fp8 out]
  U^T[dh, t] += V_h(block)^T P^T      accumulated over T-blocks in PSUM
  den        += ones8^T P8            fp8 DoubleRow: ONE matmul per
                                      PAIR of T-blocks (2x cheaper)
  U^T *= 1/max(den, tiny)             (rows with all-false mask give
                U = 0 exactly, so they stay 0 like the reference wipe)
  per head: pair AllGather of U^T, overlapped with attention
  out^T[o_own, t] = Wo_own^T U_all^T (+ bo slice), direct DMA out.

Matmul inputs are bf16 (PE 4x faster than fp32); PSUM accumulation,
softmax denominators and reciprocal stay fp32.
"""

import sys

if "/opt/trn_rl_repo" not in sys.path:
    sys.path.insert(0, "/opt/trn_rl_repo")

import numpy as np
import ml_dtypes
from contextlib import ExitStack

B, TQ, TKV, D, H = 4, 512, 4096, 1024, 8
HD = D // H            # 128 head dim
NCORES = 8
GH = H // 2            # heads per core = 4
GD = GH * HD           # 512 cols per head-group
P = 128
KC = D // P            # 8 contraction chunks
NTB = TKV // P         # 32 T-blocks
NTC = TKV // 512       # 8 T-chunks (DMA granularity)
NOB = GD // P          # 4 output blocks per core (own col half)
SCALE = float(1.0 / np.sqrt(HD))

_CACHED_NC = None


def _build_nc():
    from concourse import mybir, bacc
    from concourse.tile import TileContext

    bf = mybir.dt.bfloat16
    f8 = mybir.dt.float8e4
    f32 = mybir.dt.float32
    AF = mybir.ActivationFunctionType
    OP = mybir.AluOpType
    DR = mybir.MatmulPerfMode.DoubleRow

    nc = bacc.Bacc("TRN2", target_bir_lowering=False, debug=False,
                   num_devices=NCORES)

    # All inputs are pre-tiled on the host into partition-major layouts
    # so every DMA is 128 contiguous multi-KB descriptors.
    xqT = nc.dram_tensor("xqT", [P, KC, TQ], bf, kind="ExternalInput")
    xkvT = nc.dram_tensor("xkvT", [P, NTC, KC, 512], bf, kind="ExternalInput")
    maskT = nc.dram_tensor("maskT", [P, NTB, TQ], bf, kind="ExternalInput")
    Wq = nc.dram_tensor("Wq", [P, KC, GD], bf, kind="ExternalInput")
    Wk = nc.dram_tensor("Wk", [P, KC, GD], bf, kind="ExternalInput")
    Wv = nc.dram_tensor("Wv", [P, KC, GD], bf, kind="ExternalInput")
    Wo = nc.dram_tensor("Wo", [P, H, GD], bf, kind="ExternalInput")
    bq = nc.dram_tensor("bq", [GD], f32, kind="ExternalInput")
    bk = nc.dram_tensor("bk", [GD], f32, kind="ExternalInput")
    bv = nc.dram_tensor("bv", [GD], f32, kind="ExternalInput")
    bo = nc.dram_tensor("bo", [GD], f32, kind="ExternalInput")
    out = nc.dram_tensor("out", [P, NOB, TQ], bf, kind="ExternalOutput")

    with TileContext(nc) as tc:
        with ExitStack() as ctx:
            persist = ctx.enter_context(tc.tile_pool(name="persist", bufs=1))
            kvchunk = ctx.enter_context(tc.tile_pool(name="kvchunk", bufs=2))
            work = ctx.enter_context(tc.tile_pool(name="work", bufs=3))
            outp = ctx.enter_context(tc.tile_pool(name="outp", bufs=1))
            # One pool of double-bank [P, 2, TQ] psum tiles serves the
            # projections (using one half) and the attention S-tiles.
            ppool = ctx.enter_context(
                tc.tile_pool(name="ppool", bufs=2, space="PSUM"))
            upool = ctx.enter_context(
                tc.tile_pool(name="upool", bufs=2, space="PSUM"))
            dpool = ctx.enter_context(
                tc.tile_pool(name="dpool", bufs=2, space="PSUM"))
            dram = ctx.enter_context(
                tc.tile_pool(name="dram", bufs=1, space="DRAM"))

            # ---- constants / weights / biases -------------------------
            # DMA emission order matters for time-to-first-matmul: Wq+xq
            # first so the Q projection starts ~6us in, then Wk/Wv, then
            # the kv chunks; mask/Wo are only needed later.
            wq_sb = persist.tile([P, KC, GD], bf)
            xq_sb = persist.tile([P, KC, TQ], bf)
            for kc in range(KC):
                nc.sync.dma_start(wq_sb[:, kc:kc + 1, :], Wq.ap()[:, kc:kc + 1, :])
                nc.sync.dma_start(xq_sb[:, kc:kc + 1, :], xqT.ap()[:, kc:kc + 1, :])

            bq_sb = persist.tile([P, GH], f32)
            bk_sb = persist.tile([P, GH], f32)
            nc.sync.dma_start(bq_sb[:], bq.ap().rearrange("(h p) -> p h", p=P))
            nc.sync.dma_start(bk_sb[:], bk.ap().rearrange("(h p) -> p h", p=P))
            bv_row = persist.tile([1, GD], f32)
            nc.sync.dma_start(bv_row[:], bv.ap().unsqueeze(0))
            bv_rep = persist.tile([P, GD], f32)
            nc.gpsimd.partition_broadcast(bv_rep[:], bv_row[:])

            ones8 = persist.tile([P, 2, P], f8)
            nc.vector.memset(ones8[:], 1.0)

            wk_sb = persist.tile([P, KC, GD], bf)
            wv_sb = persist.tile([P, KC, GD], bf)
            kv_tiles = {}

            def load_kv_chunk(tcknk):
                t = kvchunk.tile([P, KC, 512], bf, name="xkv_t", tag="xkv")
                nc.sync.dma_start(t[:], xkvT.ap()[:, tcknk, :, :])
                kv_tiles[tcknk] = t

            nc.sync.dma_start(wk_sb[:], Wk.ap())
            load_kv_chunk(0)
            nc.sync.dma_start(wv_sb[:], Wv.ap())

            # ---- Q^T = Wq_g^T x_q^T  (+bq) ----------------------------
            qt_sb = persist.tile([P, GH, TQ], bf)
            for db in range(GH):
                ps = ppool.tile([P, 2, TQ], f32, name="proj_ps",
                                tag="big")[:, 0, :]
                for kc in range(KC):
                    nc.tensor.matmul(ps[:], wq_sb[:, kc, db * P:(db + 1) * P],
                                     xq_sb[:, kc, :],
                                     start=(kc == 0), stop=(kc == KC - 1))
                nc.vector.tensor_tensor(
                    qt_sb[:, db, :], ps[:],
                    bq_sb[:, db:db + 1].to_broadcast([P, TQ]), OP.add)

            # ---- K^T and V over T-chunks ------------------------------
            kt_sb = persist.tile([P, GH, TKV], bf)
            v_sb = persist.tile([P, NTB, GD], bf)
            mask_sb = persist.tile([P, NTB, TQ], bf)
            bo_sb = persist.tile([P, NOB], f32)
            wo_sb = persist.tile([P, H, GD], bf)
            for tcknk in range(NTC):
                if tcknk + 1 < NTC:
                    load_kv_chunk(tcknk + 1)
                xkv_t = kv_tiles.pop(tcknk)
                if tcknk == 1:
                    # queue the bulk "later-phase" loads behind chunks 0-1
                    nc.sync.dma_start(mask_sb[:], maskT.ap())
                    nc.sync.dma_start(wo_sb[:], Wo.ap())
                    nc.sync.dma_start(
                        bo_sb[:], bo.ap().rearrange("(ob p) -> p ob", p=P))
                for db in range(GH):
                    ps = ppool.tile([P, 2, 512], f32, name="proj_ps",
                                    tag="big")[:, 0, :]
                    for kc in range(KC):
                        nc.tensor.matmul(ps[:], wk_sb[:, kc, db * P:(db + 1) * P],
                                         xkv_t[:, kc, :],
                                         start=(kc == 0), stop=(kc == KC - 1))
                    nc.vector.tensor_tensor(
                        kt_sb[:, db, tcknk * 512:(tcknk + 1) * 512], ps[:],
                        bk_sb[:, db:db + 1].to_broadcast([P, 512]), OP.add)
                for tb in range(4):
                    ps = ppool.tile([P, 2, 512], f32, name="proj_ps",
                                    tag="big")[:, 0, :]
                    for kc in range(KC):
                        nc.tensor.matmul(ps[:],
                                         xkv_t[:, kc, tb * P:(tb + 1) * P],
                                         wv_sb[:, kc, :],
                                         start=(kc == 0), stop=(kc == KC - 1))
                    nc.vector.tensor_tensor(
                        v_sb[:, tcknk * 4 + tb, :], ps[:], bv_rep[:], OP.add)

            # ---- attention, flattened double-step loop ----------------
            # Two T-blocks per step: two S-matmuls fill the two banks of
            # one [P, 2, TQ] psum tile, then ONE wide exp (ACT per-op
            # overhead amortized below the PE pace), one wide mask-mult
            # (bf16, feeds U) and one wide mask-mult to fp8 (feeds den).
            ut_sb = persist.tile([P, GH, TQ], bf)
            u_all = persist.tile([P, 2, GH, TQ], bf)
            cc_in012 = dram.tile([3, P, TQ], bf, name="cc_in012")
            cc_out012 = dram.tile([2, 3, P, TQ], bf, name="cc_out012")
            cc_in3 = dram.tile([P, TQ], bf, name="cc_in3")
            cc_out3 = dram.tile([2, P, TQ], bf, name="cc_out3")
            RG = [[0, 1], [2, 3], [4, 5], [6, 7]]

            NDS = GH * NTB // 2
            s_tiles = {}
            u_tiles = [None] * GH
            den_tiles = [None] * GH
            SPRE = 2  # double-step prefetch depth

            def s2_mm(ds):
                t2 = ppool.tile([P, 2, TQ], f32, name="s2_ps", tag="big")
                for k in range(2):
                    h, j = divmod(ds * 2 + k, NTB)
                    nc.tensor.matmul(t2[:, k, :],
                                     kt_sb[:, h, j * P:(j + 1) * P],
                                     qt_sb[:, h, :], start=True, stop=True)
                return t2

            deferred = {}
            fstate = {}

            def fin_max(h):
                den_sf = work.tile([1, TQ], f32, tag="den_sf", bufs=2)
                nc.vector.tensor_scalar(den_sf[:], den_tiles[h][0:1, :],
                                        1e-30, None, OP.max)
                fstate[h] = [den_sf]

            def fin_recip(h):
                den_sf, = fstate[h]
                recip = work.tile([1, TQ], f32, tag="recip", bufs=2)
                nc.vector.reciprocal(recip[:], den_sf[:])
                recip_rep = work.tile([P, TQ], f32, tag="recip_rep", bufs=2)
                nc.gpsimd.partition_broadcast(recip_rep[:], recip[:])
                fstate[h] = [recip_rep]

            def fin_scale_exch(h):
                recip_rep, = fstate.pop(h)
                nc.vector.tensor_tensor(ut_sb[:, h, :], u_tiles[h][:],
                                        recip_rep[:], OP.mult)
                # pair-exchange U while attention continues: heads 0-2 go
                # in one batched AllGather launched at head 2, head 3 in a
                # final small one overlapped with the stage-A out-proj.
                if h < 3:
                    nc.sync.dma_start(cc_in012[h], ut_sb[:, h, :])
                else:
                    nc.sync.dma_start(cc_in3[:], ut_sb[:, h, :])
                if h == 2:
                    nc.gpsimd.collective_compute(
                        "AllGather", OP.bypass, replica_groups=RG,
                        ins=[cc_in012.opt()], outs=[cc_out012.opt()])
                    for r in range(2):
                        nc.sync.dma_start(
                            u_all[:, r, 0:3, :],
                            cc_out012[r].rearrange("h p t -> p h t"))
                elif h == 3:
                    nc.gpsimd.collective_compute(
                        "AllGather", OP.bypass, replica_groups=RG,
                        ins=[cc_in3.opt()], outs=[cc_out3.opt()])
                    nc.sync.dma_start(
                        u_all[:, :, 3, :],
                        cc_out3[:].rearrange("r p t -> p r t"))

            p_tiles = {}
            p8_tiles = {}
            for pre in range(SPRE):
                s_tiles[pre] = s2_mm(pre)
            # U/den run one double-step behind exp/mask so their moving
            # operand is always ready when they reach the PE queue head.
            for it in range(NDS + 1):
                for fn in deferred.pop(it, []):
                    fn()
                if it < NDS:
                    h, j0 = divmod(it * 2, NTB)
                    t2 = s_tiles.pop(it)
                    praw = work.tile([P, 2, TQ], bf, tag="praw", bufs=2)
                    nc.scalar.activation(praw[:], t2[:], AF.Exp, scale=SCALE)
                    p_t = work.tile([P, 2, TQ], bf, tag="p_t", bufs=3)
                    nc.vector.tensor_tensor(p_t[:], praw[:],
                                            mask_sb[:, j0:j0 + 2, :], OP.mult)
                    p8 = work.tile([P, 2, TQ], f8, tag="p8", bufs=3)
                    nc.vector.tensor_tensor(p8[:], praw[:],
                                            mask_sb[:, j0:j0 + 2, :], OP.mult)
                    p_tiles[it] = p_t
                    p8_tiles[it] = p8
                    if it + SPRE < NDS:
                        s_tiles[it + SPRE] = s2_mm(it + SPRE)
                if it >= 1:
                    dsu = it - 1
                    h, j0 = divmod(dsu * 2, NTB)
                    if j0 == 0:
                        u_tiles[h] = upool.tile([P, TQ], f32, name="u_ps",
                                                tag="u_ps")
                        den_tiles[h] = dpool.tile([P, TQ], f32, name="den_ps",
                                                  tag="den_ps")
                    p_t = p_tiles.pop(dsu)
                    p8 = p8_tiles.pop(dsu)
                    for k in range(2):
                        j = j0 + k
                        nc.tensor.matmul(u_tiles[h][:],
                                         v_sb[:, j, h * P:(h + 1) * P],
                                         p_t[:, k, :],
                                         start=(j == 0), stop=(j == NTB - 1))
                    nc.tensor.matmul(den_tiles[h][:], ones8[:], p8[:],
                                     start=(j0 == 0), stop=(j0 + 2 == NTB),
                                     perf_mode=DR)
                    if j0 + 2 == NTB:
                        # Defer the reciprocal chain a few double-steps
                        # into the next head so its DVE ops don't delay
                        # the mask-mults that feed the U matmuls.
                        deferred.setdefault(it + 1, []).append(
                            lambda h=h: fin_max(h))
                        deferred.setdefault(it + 2, []).append(
                            lambda h=h: fin_recip(h))
                        deferred.setdefault(it + 3, []).append(
                            lambda h=h: fin_scale_exch(h))

            for ds_late in sorted(deferred):
                for fn in deferred.pop(ds_late):
                    fn()

            # ---- out cols [g*512,(g+1)*512) = Wo_own^T U_all (+bo) ----
            # Stage A: heads 0-2 of both ranks (AG012 already landed)
            # overlaps the in-flight AG3; stage B adds heads 3/7.
            o_sb = outp.tile([P, NOB, TQ], bf)
            o_ps = [ppool.tile([P, 2, TQ], f32, name="o_ps", tag="big")
                    for _ in range(2)]

            def ops(ob):
                return o_ps[ob // 2][:, ob % 2, :]

            for ob in range(NOB):
                for idx, hh in enumerate((0, 1, 2, 4, 5, 6)):
                    r, lh = divmod(hh, GH)
                    nc.tensor.matmul(ops(ob),
                                     wo_sb[:, hh, ob * P:(ob + 1) * P],
                                     u_all[:, r, lh, :],
                                     start=(idx == 0), stop=False)
            for ob in range(NOB):
                for hh in (3, 7):
                    r, lh = divmod(hh, GH)
                    nc.tensor.matmul(ops(ob),
                                     wo_sb[:, hh, ob * P:(ob + 1) * P],
                                     u_all[:, r, lh, :],
                                     start=False, stop=(hh == 7))
                nc.vector.tensor_tensor(
                    o_sb[:, ob, :], ops(ob),
                    bo_sb[:, ob:ob + 1].to_broadcast([P, TQ]), OP.add)
                nc.sync.dma_start(out.ap()[:, ob:ob + 1, :],
                                  o_sb[:, ob:ob + 1, :])

    nc.finalize()
    return nc


def _ptile(a2d, inner):
    """[R, C] row-major -> [P, R//P, C] partition-major, contiguous."""
    r, c = a2d.shape
    return np.ascontiguousarray(
        a2d.reshape(r // P, P, c).transpose(1, 0, 2)).astype(inner)


def _shard_inputs(inputs_q, inputs_kv, attention_mask, Wq, bq, Wk, bk, Wv, bv,
                  Wo, bo):
    bf16 = ml_dtypes.bfloat16
    f32 = np.float32

    xqT = [_ptile(inputs_q[b].T, bf16) for b in range(B)]         # [P,KC,TQ]
    xkvT = [_ptile(inputs_kv[b].T, bf16)                          # [P,NTC,KC,512]
            .reshape(P, KC, NTC, 512).transpose(0, 2, 1, 3).copy()
            for b in range(B)]
    maskT = [_ptile(attention_mask[b].T.astype(np.float32), bf16)  # [P,NTB,TQ]
             for b in range(B)]
    in_maps = []
    for c in range(NCORES):
        b, g = c // 2, c % 2  # pair = (2b, 2b+1)
        sl = slice(g * GD, (g + 1) * GD)
        in_maps.append({
            "xqT": xqT[b],
            "xkvT": xkvT[b],
            "maskT": maskT[b],
            "Wq": _ptile(np.ascontiguousarray(Wq[:, sl]), bf16),
            "Wk": _ptile(np.ascontiguousarray(Wk[:, sl]), bf16),
            "Wv": _ptile(np.ascontiguousarray(Wv[:, sl]), bf16),
            # all head rows x own col half, [P, H, GD] bf16
            "Wo": _ptile(np.ascontiguousarray(Wo[:, sl]), bf16),
            "bq": np.ascontiguousarray(bq[sl]).astype(f32),
            "bk": np.ascontiguousarray(bk[sl]).astype(f32),
            "bv": np.ascontiguousarray(bv[sl]).astype(f32),
            "bo": np.ascontiguousarray(bo[sl]).astype(f32),
        })
    return in_maps


def kernel(_trace=False, **inputs):
    global _CACHED_NC
    from concourse import bass_utils

    arrs = {k: np.asarray(v) for k, v in inputs.items()}
    in_maps = _shard_inputs(**arrs)

    if _CACHED_NC is None:
        _CACHED_NC = _build_nc()

    res = bass_utils.run_bass_kernel_spmd(
        _CACHED_NC, in_maps, core_ids=list(range(NCORES)), trace=_trace)

    full = np.empty((B, TQ, D), np.float32)
    for c in range(NCORES):
        b, g = c // 2, c % 2
        o = res.results[c]["out"]  # [P, NOB, TQ] bf16, o-col = ob*128+p
        full[b][:, g * GD:(g + 1) * GD] = (
            o.transpose(2, 1, 0).reshape(TQ, GD).astype(np.float32))
    if _trace:
        return full, res
    return full
